# revision 1
# baseline (speedup 1.0000x reference)
"""CRX gate (controlled-RX on 12-qubit state batch) as a Trainium2 Bass kernel.

Problem: y = U @ x with U the CRX(angle) unitary, DIM=2, NQ=12, control
qubit 0 (stride 2048), target qubit 1 (stride 1024), D=4096, B=128.

Semantics (derived from the reference):
  - rows d in [0, 2048): control bit 0 -> identity (y = x)
  - rows d in [2048, 3072) pair with d+1024; with c=cos(angle/2),
    s=sin(angle/2):
      y[d]      = c*x[d]      - 1j*s*x[d+1024]
      y[d+1024] = -1j*s*x[d]  + c*x[d+1024]

Strategy: batch (column) sharding across 8 NeuronCores, 16 columns each
(data parallel over the 128 states, per the sharding hint; U is never
materialized). Only the rotated half (rows 2048:4096) is shipped to the
device; the identity half is a host passthrough.

Per core the device sees one [128, 772] f32 tile:
  cols 0:4    (c, s, -, pad) replicated per partition -- the NEFF is
              angle-independent and compiled exactly once per process
  cols 4:516  X = [L | R], L = [Ar | Br], R = [Bi | Ai]
              (A = rows 2048:3072, B = rows 3072:4096, r/i = real/imag)
  cols 516:772  -L (host-negated copy of L)
With that packing the whole rotate is TWO big contiguous DVE ops:
  t = c * X                      (tensor_scalar_mul, [128, 512])
  o = s * X[256:768] + t         (ONE fused scalar_tensor_tensor whose
                                  in0 = [R | -L], so out = [c*L + s*R |
                                  c*R - s*L] with a uniform +s scalar)
The sign trick ((-L)*s == L*(-s) exactly in f32) keeps the result
bit-identical to the reference while folding both output halves into a
single contiguous instruction.

Structure chosen by drift-controlled rep-slope wall-clock measurement
(the NTFF profiling hook is unavailable under this axon container; the
backend charges ~30 us fixed per DMA and ~22 us fixed per DVE op, so
minimal-instruction single-DMA structures beat every chunked /
multi-engine / shear variant; the 2-op compute beat the 3-op pair
layout by ~15%).

Raw Bass (no TileContext): the Tile tail drain accumulates >1 sem wait,
which this container's walrus codegen rejects ("Too many sync wait
commands"), so synchronization is manual: completion is signalled via
then_inc on the final instruction of each stage.
"""

import numpy as np

_NCORES = 8
_D = 4096
_B = 128
_BC = _B // _NCORES  # 16 batch columns per core
_HALF = 2048
_Q = 1024
_W = 512             # output data columns per core
_S = 4               # leading scalar/pad columns: c, s, unused, pad
_XW = _S + _W + _W // 2  # input tile width: scalars + X + (-L) = 772

LAST_RESULTS = None   # BassKernelResults of the most recent run (for test.py)
_NC_CACHE = None      # angle-independent Bass module, built once per process


def _build_bass():
    import concourse.bass as bass
    import concourse.mybir as mybir

    MUL, ADD = mybir.AluOpType.mult, mybir.AluOpType.add

    nc = bass.Bass("TRN2")
    x = nc.dram_tensor("x", [128, _XW], mybir.dt.float32, kind="ExternalInput")
    y = nc.dram_tensor("y", [128, _W], mybir.dt.float32, kind="ExternalOutput")

    with (
        nc.sbuf_tensor([128, _XW], mybir.dt.float32) as xt,
        nc.sbuf_tensor([128, _W], mybir.dt.float32) as t,
        nc.sbuf_tensor([128, _W], mybir.dt.float32) as o,
        nc.semaphore() as dsem_in,
        nc.semaphore() as vsem,
        nc.semaphore() as dsem_out,
        nc.Block() as block,
    ):
        cv = xt[:, 0:1]   # c per partition
        sv = xt[:, 1:2]   # s

        @block.sync
        def _(sync):
            sync.dma_start(xt[:], x[:]).then_inc(dsem_in, 16)
            sync.wait_ge(vsem, 1)
            sync.dma_start(y[:], o[:]).then_inc(dsem_out, 16)
            sync.wait_ge(dsem_out, 16)

        @block.vector
        def _(vector):
            vector.wait_ge(dsem_in, 16)
            nc.vector.tensor_scalar_mul(t[:], xt[:, _S : _S + _W], cv)
            # in0 = cols [R | -L]; out = [s*R + c*L | s*(-L) + c*R]
            nc.vector.scalar_tensor_tensor(
                out=o[:],
                in0=xt[:, _S + _W // 2 : _XW],
                scalar=sv,
                in1=t[:],
                op0=MUL,
                op1=ADD,
            ).then_inc(vsem, 1)

    return nc


def _get_nc():
    global _NC_CACHE
    if _NC_CACHE is None:
        _NC_CACHE = _build_bass()
    return _NC_CACHE


def _prep_in_maps(x: np.ndarray, c: float, s: float):
    A = x[_HALF : _HALF + _Q]  # (1024, 128)
    Bv = x[_HALF + _Q :]       # (1024, 128)
    half = _W // 2
    in_maps = []
    for k in range(_NCORES):
        sl = slice(k * _BC, (k + 1) * _BC)
        M = np.stack(
            [A[:, sl].real, Bv[:, sl].real, Bv[:, sl].imag, A[:, sl].imag]
        )  # (4, 1024, BC) f32 -- quarters [Ar | Br | Bi | Ai] = [L | R]
        Xk = np.empty((128, _XW), dtype=np.float32)
        Xk[:, 0] = c
        Xk[:, 1] = s
        Xk[:, 2] = 0.0
        Xk[:, 3] = 0.0
        # row d' = n*128 + p -> [p, quarter*128 + n*16 + b]
        Xk[:, _S : _S + _W] = (
            M.reshape(4, 8, 128, _BC).transpose(2, 0, 1, 3).reshape(128, _W)
        )
        np.negative(Xk[:, _S : _S + half], out=Xk[:, _S + _W :])  # -L
        in_maps.append({"x": Xk})
    return in_maps


def _unpack_out(y: np.ndarray, results):
    for k in range(_NCORES):
        sl = slice(k * _BC, (k + 1) * _BC)
        Yk = (
            results[k]["y"]
            .reshape(128, 4, 8, _BC)
            .transpose(1, 2, 0, 3)
            .reshape(4, _Q, _BC)
        )  # quarters [Ar' | Br' | Bi' | Ai']
        y[_HALF : _HALF + _Q, sl] = Yk[0] + 1j * Yk[3]
        y[_HALF + _Q :, sl] = Yk[1] + 1j * Yk[2]


def kernel(x, angle):
    global LAST_RESULTS
    from concourse.bass_utils import run_bass_kernel_spmd

    x = np.asarray(x)
    angle = np.asarray(angle)
    assert x.shape == (_D, _B), x.shape
    if x.dtype != np.complex64:
        x = x.astype(np.complex64)

    theta = 0.5 * float(np.float32(angle.reshape(-1)[0]))
    c = float(np.cos(theta))
    s = float(np.sin(theta))

    y = np.empty((_D, _B), dtype=np.complex64)
    y[:_HALF] = x[:_HALF]  # control bit 0: identity

    in_maps = _prep_in_maps(x, c, s)
    nc = _get_nc()
    res = run_bass_kernel_spmd(nc, in_maps, core_ids=list(range(_NCORES)))
    LAST_RESULTS = res
    _unpack_out(y, res.results)
    return y



# revision 7
# speedup vs baseline: 1.1853x; 1.1853x over previous
"""CRX gate (controlled-RX on 12-qubit state batch) as a Trainium2 Bass kernel.

Problem: y = U @ x with U the CRX(angle) unitary, DIM=2, NQ=12, control
qubit 0 (stride 2048), target qubit 1 (stride 1024), D=4096, B=128.

Semantics (derived from the reference):
  - rows d in [0, 2048): control bit 0 -> identity (y = x)
  - rows d in [2048, 3072) pair with d+1024; with c=cos(angle/2),
    s=sin(angle/2), A = x[2048:3072], B = x[3072:4096]:
      yA.r = c*A.r + s*B.i      yA.i = c*A.i - s*B.r
      yB.r = c*B.r + s*A.i      yB.i = c*B.i - s*A.r

Strategy: batch (column) sharding across 8 NeuronCores, 16 columns each
(data parallel, per the sharding hint; U is never materialized). Only the
rotated half ships to the device; the identity half is a host passthrough.

The rotate decomposes into two independent quarter-pair blocks:
  P block: (q0=A.r, q2=B.i) -> (yA.r, yB.i)
  Q block: (q1=B.r, q3=A.i) -> (yB.r, yA.i)
Each block is one fast fp16 tensor_scalar_mul (4x DVE mode) plus two
scalar_tensor_tensor ops. All data moves as float16 (rel-err budget 2e-2;
fp16 contributes ~5e-4), halving DMA transfer time vs the f32 baseline.

Timing structure (per the concourse TimelineSim cost model): the P and Q
blocks get separate input and output DMAs, all issued from the SP queue.
P's input lands ~600ns before Q's, so P's compute overlaps Q's input
sem-propagation, and P's output DMA (HWDGE+DGE latency ~1275ns) overlaps
Q's compute + Q's output HWDGE. The prepared-SWDGE trigger path
(dma_scatter_add prepare_only + trigger_dma) would cut another ~1.3us of
post-compute HWDGE latency but this container's runtime ucode cannot
execute the extended GPSIMD DMA instructions (device-unrecoverable on
HW), so plain HWDGE dma_starts are used throughout.

Raw Bass (no TileContext): the Tile tail drain accumulates >1 sem wait,
which this container's walrus codegen rejects ("Too many sync wait
commands"), so synchronization is manual. Waits are fused onto consuming
instructions (._wait_ge) so instruction decode happens inside wait
windows instead of serializing after them.
"""

import numpy as np

_NCORES = 8
_D = 4096
_B = 128
_BC = _B // _NCORES  # 16 batch columns per core
_HALF = 2048
_Q = 1024
_S = 8               # leading fp16 slots: c, s, -s as packed fp32 + pad
_QC = 128            # columns per quarter tile
_PW = _S + 2 * _QC   # P-block input width (scalars + q0 + q2) = 264
_QW = 2 * _QC        # Q-block input width (q1 + q3) = 256

LAST_RESULTS = None   # BassKernelResults of the most recent run (for test.py)
_NC_CACHE = None      # angle-independent Bass module, built once per process


def _build_bass():
    import concourse.bass as bass
    import concourse.mybir as mybir

    MUL, ADD = mybir.AluOpType.mult, mybir.AluOpType.add
    f16, f32 = mybir.dt.float16, mybir.dt.float32

    nc = bass.Bass("TRN2")
    xP = nc.dram_tensor("xP", [128, _PW], f16, kind="ExternalInput")
    xQ = nc.dram_tensor("xQ", [128, _QW], f16, kind="ExternalInput")
    y0 = nc.dram_tensor("y0", [128, _QW], f16, kind="ExternalOutput")  # [yA.r|yB.i]
    y1 = nc.dram_tensor("y1", [128, _QW], f16, kind="ExternalOutput")  # [yB.r|yA.i]

    with (
        nc.sbuf_tensor([128, _PW], f16) as xtP,
        nc.sbuf_tensor([128, _QW], f16) as xtQ,
        nc.sbuf_tensor([128, _QW], f16) as tP,
        nc.sbuf_tensor([128, _QW], f16) as tQ,
        nc.sbuf_tensor([128, _QW], f16) as oP,
        nc.sbuf_tensor([128, _QW], f16) as oQ,
        nc.semaphore() as dsemP,
        nc.semaphore() as dsemQ,
        nc.semaphore() as sP,
        nc.semaphore() as sQ,
        nc.semaphore() as dout,
        nc.Block() as block,
    ):
        cv = xtP[:, 0:2].bitcast(f32)   # [128, 1] fp32 views of packed scalars
        sv = xtP[:, 2:4].bitcast(f32)
        nv = xtP[:, 4:6].bitcast(f32)
        q0 = xtP[:, _S : _S + _QC]
        q2 = xtP[:, _S + _QC : _S + 2 * _QC]
        q1 = xtQ[:, 0:_QC]
        q3 = xtQ[:, _QC : 2 * _QC]

        @block.sync
        def _(sync):
            sync.dma_start(xtP[:], xP[:]).then_inc(dsemP, 16)
            sync.dma_start(xtQ[:], xQ[:]).then_inc(dsemQ, 16)
            sync.dma_start(y0[:], oP[:]).then_inc(dout, 16)._wait_ge(sP, 1)
            sync.dma_start(y1[:], oQ[:]).then_inc(dout, 16)._wait_ge(sQ, 1)
            sync.wait_ge(dout, 32)

        @block.vector
        def _(vector):
            # P block: waits fused onto the ops so decode happens early
            nc.vector.tensor_scalar_mul(tP[:], xtP[:, _S:_PW], cv)._wait_ge(dsemP, 16)
            nc.vector.scalar_tensor_tensor(
                out=oP[:, 0:_QC], in0=q2, scalar=sv, in1=tP[:, 0:_QC],
                op0=MUL, op1=ADD,
            )
            nc.vector.scalar_tensor_tensor(
                out=oP[:, _QC:], in0=q0, scalar=nv, in1=tP[:, _QC:],
                op0=MUL, op1=ADD,
            ).then_inc(sP, 1)
            # Q block
            nc.vector.tensor_scalar_mul(tQ[:], xtQ[:], cv)._wait_ge(dsemQ, 16)
            nc.vector.scalar_tensor_tensor(
                out=oQ[:, 0:_QC], in0=q3, scalar=sv, in1=tQ[:, 0:_QC],
                op0=MUL, op1=ADD,
            )
            nc.vector.scalar_tensor_tensor(
                out=oQ[:, _QC:], in0=q1, scalar=nv, in1=tQ[:, _QC:],
                op0=MUL, op1=ADD,
            ).then_inc(sQ, 1)

    return nc


def _get_nc():
    global _NC_CACHE
    if _NC_CACHE is None:
        _NC_CACHE = _build_bass()
    return _NC_CACHE


def _to_tile(q):
    # (1024, 16) -> [128, 128]: row d' = n*128 + p -> [p, n*16 + b]
    return q.reshape(8, 128, _BC).transpose(1, 0, 2).reshape(128, _QC)


def _from_tile(t):
    # inverse of _to_tile
    return t.reshape(128, 8, _BC).transpose(1, 0, 2).reshape(_Q, _BC)


def _prep_in_maps(x, c, s):
    A = x[_HALF : _HALF + _Q]  # (1024, 128) complex64
    Bv = x[_HALF + _Q :]
    scal = np.array([c, s, -s, 0.0], dtype=np.float32).view(np.float16)  # 8 slots
    in_maps = []
    for k in range(_NCORES):
        sl = slice(k * _BC, (k + 1) * _BC)
        XP = np.empty((128, _PW), dtype=np.float16)
        XP[:, 0:_S] = scal
        XP[:, _S : _S + _QC] = _to_tile(A[:, sl].real.astype(np.float16))    # q0
        XP[:, _S + _QC :] = _to_tile(Bv[:, sl].imag.astype(np.float16))      # q2
        XQ = np.empty((128, _QW), dtype=np.float16)
        XQ[:, 0:_QC] = _to_tile(Bv[:, sl].real.astype(np.float16))           # q1
        XQ[:, _QC:] = _to_tile(A[:, sl].imag.astype(np.float16))             # q3
        in_maps.append({"xP": XP, "xQ": XQ})
    return in_maps


def _unpack_out(y, results):
    for k in range(_NCORES):
        sl = slice(k * _BC, (k + 1) * _BC)
        r0 = results[k]["y0"].astype(np.float32)  # [yA.r | yB.i]
        r1 = results[k]["y1"].astype(np.float32)  # [yB.r | yA.i]
        yAr = _from_tile(r0[:, 0:_QC])
        yBi = _from_tile(r0[:, _QC:])
        yBr = _from_tile(r1[:, 0:_QC])
        yAi = _from_tile(r1[:, _QC:])
        y[_HALF : _HALF + _Q, sl] = yAr + 1j * yAi
        y[_HALF + _Q :, sl] = yBr + 1j * yBi


def kernel(x, angle):
    global LAST_RESULTS
    from concourse.bass_utils import run_bass_kernel_spmd

    x = np.asarray(x)
    angle = np.asarray(angle)
    assert x.shape == (_D, _B), x.shape
    if x.dtype != np.complex64:
        x = x.astype(np.complex64)

    theta = 0.5 * float(np.float32(angle.reshape(-1)[0]))
    c = float(np.cos(theta))
    s = float(np.sin(theta))

    y = np.empty((_D, _B), dtype=np.complex64)
    y[:_HALF] = x[:_HALF]  # control bit 0: identity

    in_maps = _prep_in_maps(x, c, s)
    nc = _get_nc()
    res = run_bass_kernel_spmd(nc, in_maps, core_ids=list(range(_NCORES)))
    LAST_RESULTS = res
    _unpack_out(y, res.results)
    return y


# revision 12
# speedup vs baseline: 1.3167x; 1.1109x over previous
"""CRX gate (controlled-RX on 12-qubit state batch) as a Trainium2 Bass kernel.

Problem: y = U @ x with U the CRX(angle) unitary, DIM=2, NQ=12, control
qubit 0 (stride 2048), target qubit 1 (stride 1024), D=4096, B=128.

Semantics (derived from the reference):
  - rows d in [0, 2048): control bit 0 -> identity (y = x)
  - rows d in [2048, 3072) pair with d+1024; with c=cos(angle/2),
    s=sin(angle/2), A = x[2048:3072], B = x[3072:4096]:
      yA.r = c*A.r + s*B.i      yA.i = c*A.i - s*B.r
      yB.r = c*B.r + s*A.i      yB.i = c*B.i - s*A.r

Strategy: batch (column) sharding across 8 NeuronCores, 16 columns each
(data parallel, per the sharding hint; U is never materialized). Only the
rotated half ships to the device; the identity half is a host passthrough.

The rotate decomposes into two independent quarter-pair blocks:
  P block: (q0=A.r, q2=B.i) -> (yA.r, yB.i)
  Q block: (q1=B.r, q3=A.i) -> (yB.r, yA.i)
Each block is one fast fp16 tensor_scalar_mul (4x DVE mode) plus two
scalar_tensor_tensor ops. All data moves as float16 (rel-err budget 2e-2;
fp16 contributes ~5e-4), halving DMA transfer time vs the f32 baseline.

Timing structure (per the concourse TimelineSim cost model): the P and Q
blocks get separate input and output DMAs, all issued from the SP queue.
P's input lands ~600ns before Q's, so P's compute overlaps Q's input
sem-propagation, and P's output DMA (HWDGE+DGE latency ~1275ns) overlaps
Q's compute + Q's output HWDGE. The schedule is balanced: the end time is
simultaneously bound by sP + 2xHWDGE + DGE + T + semprop and by
sQ + HWDGE + DGE + T + semprop. The prepared-SWDGE trigger path
(dma_scatter_add prepare_only + trigger_dma) would cut another ~1.3us of
post-compute HWDGE latency — it compiles once codegen_inst_isa_subclasses
populates the .instr bytes — but this container's runtime ucode cannot
execute the extended GPSIMD DMA instructions (device-unrecoverable on
HW), so plain HWDGE dma_starts are used throughout.

Raw Bass (no TileContext): the Tile tail drain accumulates >1 sem wait,
which this container's walrus codegen rejects ("Too many sync wait
commands"), so synchronization is manual. Waits are fused onto consuming
instructions (._wait_ge) so instruction decode happens inside wait
windows instead of serializing after them.
"""

import numpy as np

_NCORES = 8
_D = 4096
_B = 128
_BC = _B // _NCORES  # 16 batch columns per core
_HALF = 2048
_Q = 1024
_S = 8               # leading fp16 slots: c, s, -s as packed fp32 + pad
_QC = 128            # columns per quarter tile
_PW = _S + 2 * _QC   # P-block input width (scalars + q0 + q2) = 264
_QW = 2 * _QC        # Q-block input width (q1 + q3) = 256

LAST_RESULTS = None   # BassKernelResults of the most recent run (for test.py)
_NC_CACHE = None      # angle-independent Bass module, built once per process


def _build_bass():
    import concourse.bass as bass
    import concourse.mybir as mybir

    MUL, ADD = mybir.AluOpType.mult, mybir.AluOpType.add
    f16, f32 = mybir.dt.float16, mybir.dt.float32

    nc = bass.Bass("TRN2")
    xP = nc.dram_tensor("xP", [128, _PW], f16, kind="ExternalInput")
    xQ = nc.dram_tensor("xQ", [128, _QW], f16, kind="ExternalInput")
    y0 = nc.dram_tensor("y0", [128, _QW], f16, kind="ExternalOutput")  # [yA.r|yB.i]
    y1 = nc.dram_tensor("y1", [128, _QW], f16, kind="ExternalOutput")  # [yB.r|yA.i]

    with (
        nc.sbuf_tensor([128, _PW], f16) as xtP,
        nc.sbuf_tensor([128, _QW], f16) as xtQ,
        nc.sbuf_tensor([128, _QW], f16) as tP,
        nc.sbuf_tensor([128, _QW], f16) as tQ,
        nc.sbuf_tensor([128, _QW], f16) as oP,
        nc.sbuf_tensor([128, _QW], f16) as oQ,
        nc.semaphore() as dsemP,
        nc.semaphore() as dsemQ,
        nc.semaphore() as sP,
        nc.semaphore() as sQ,
        nc.semaphore() as dout,
        nc.Block() as block,
    ):
        cv = xtP[:, 0:2].bitcast(f32)   # [128, 1] fp32 views of packed scalars
        sv = xtP[:, 2:4].bitcast(f32)
        nv = xtP[:, 4:6].bitcast(f32)
        q0 = xtP[:, _S : _S + _QC]
        q2 = xtP[:, _S + _QC : _S + 2 * _QC]
        q1 = xtQ[:, 0:_QC]
        q3 = xtQ[:, _QC : 2 * _QC]

        @block.sync
        def _(sync):
            sync.dma_start(xtP[:], xP[:]).then_inc(dsemP, 16)
            sync.dma_start(xtQ[:], xQ[:]).then_inc(dsemQ, 16)
            sync.dma_start(y0[:], oP[:]).then_inc(dout, 16)._wait_ge(sP, 1)
            sync.dma_start(y1[:], oQ[:]).then_inc(dout, 16)._wait_ge(sQ, 1)
            sync.wait_ge(dout, 32)

        @block.vector
        def _(vector):
            # P block: waits fused onto the ops so decode happens early
            nc.vector.tensor_scalar_mul(tP[:], xtP[:, _S:_PW], cv)._wait_ge(dsemP, 16)
            nc.vector.scalar_tensor_tensor(
                out=oP[:, 0:_QC], in0=q2, scalar=sv, in1=tP[:, 0:_QC],
                op0=MUL, op1=ADD,
            )
            nc.vector.scalar_tensor_tensor(
                out=oP[:, _QC:], in0=q0, scalar=nv, in1=tP[:, _QC:],
                op0=MUL, op1=ADD,
            ).then_inc(sP, 1)
            # Q block
            nc.vector.tensor_scalar_mul(tQ[:], xtQ[:], cv)._wait_ge(dsemQ, 16)
            nc.vector.scalar_tensor_tensor(
                out=oQ[:, 0:_QC], in0=q3, scalar=sv, in1=tQ[:, 0:_QC],
                op0=MUL, op1=ADD,
            )
            nc.vector.scalar_tensor_tensor(
                out=oQ[:, _QC:], in0=q1, scalar=nv, in1=tQ[:, _QC:],
                op0=MUL, op1=ADD,
            ).then_inc(sQ, 1)

    _prune_dead_preamble(nc)
    return nc


def _prune_dead_preamble(nc):
    """Drop dead preamble instructions: the four const-AP memsets on Pool
    (this kernel never reads the const APs — the BIR verifier itself flags
    them as reader-less) and every engine's register-init moves (no
    instruction in this program reads a GPR: all operands are SBUF access
    patterns).  The Pool memsets are the tail of the preamble, so the
    all-engine barrier — and with it every engine's program — starts
    ~730ns earlier.

    The preamble/epilogue barriers, drains, and block structure are kept:
    removing them simulates faster still, but soak-testing showed rare
    device-wedging crashes (engine halt racing runtime init/teardown), so
    only provably-dead instructions are pruned.  This level soaked clean
    across many repeated hardware executions."""
    import concourse.mybir as mybir

    blk = nc.m.functions[0].blocks[0]
    blk.instructions[:] = [
        i
        for i in blk.instructions
        if i.opcode not in ("Memset", "RegisterMove")
    ]


def _get_nc():
    global _NC_CACHE
    if _NC_CACHE is None:
        _NC_CACHE = _build_bass()
    return _NC_CACHE


def _to_tile(q):
    # (1024, 16) -> [128, 128]: row d' = n*128 + p -> [p, n*16 + b]
    return q.reshape(8, 128, _BC).transpose(1, 0, 2).reshape(128, _QC)


def _from_tile(t):
    # inverse of _to_tile
    return t.reshape(128, 8, _BC).transpose(1, 0, 2).reshape(_Q, _BC)


def _prep_in_maps(x, c, s):
    A = x[_HALF : _HALF + _Q]  # (1024, 128) complex64
    Bv = x[_HALF + _Q :]
    scal = np.array([c, s, -s, 0.0], dtype=np.float32).view(np.float16)  # 8 slots
    in_maps = []
    for k in range(_NCORES):
        sl = slice(k * _BC, (k + 1) * _BC)
        XP = np.empty((128, _PW), dtype=np.float16)
        XP[:, 0:_S] = scal
        XP[:, _S : _S + _QC] = _to_tile(A[:, sl].real.astype(np.float16))    # q0
        XP[:, _S + _QC :] = _to_tile(Bv[:, sl].imag.astype(np.float16))      # q2
        XQ = np.empty((128, _QW), dtype=np.float16)
        XQ[:, 0:_QC] = _to_tile(Bv[:, sl].real.astype(np.float16))           # q1
        XQ[:, _QC:] = _to_tile(A[:, sl].imag.astype(np.float16))             # q3
        in_maps.append({"xP": XP, "xQ": XQ})
    return in_maps


def _unpack_out(y, results):
    for k in range(_NCORES):
        sl = slice(k * _BC, (k + 1) * _BC)
        r0 = results[k]["y0"].astype(np.float32)  # [yA.r | yB.i]
        r1 = results[k]["y1"].astype(np.float32)  # [yB.r | yA.i]
        yAr = _from_tile(r0[:, 0:_QC])
        yBi = _from_tile(r0[:, _QC:])
        yBr = _from_tile(r1[:, 0:_QC])
        yAi = _from_tile(r1[:, _QC:])
        y[_HALF : _HALF + _Q, sl] = yAr + 1j * yAi
        y[_HALF + _Q :, sl] = yBr + 1j * yBi


def kernel(x, angle):
    global LAST_RESULTS
    from concourse.bass_utils import run_bass_kernel_spmd

    x = np.asarray(x)
    angle = np.asarray(angle)
    assert x.shape == (_D, _B), x.shape
    if x.dtype != np.complex64:
        x = x.astype(np.complex64)

    theta = 0.5 * float(np.float32(angle.reshape(-1)[0]))
    c = float(np.cos(theta))
    s = float(np.sin(theta))

    y = np.empty((_D, _B), dtype=np.complex64)
    y[:_HALF] = x[:_HALF]  # control bit 0: identity

    in_maps = _prep_in_maps(x, c, s)
    nc = _get_nc()
    res = run_bass_kernel_spmd(nc, in_maps, core_ids=list(range(_NCORES)))
    LAST_RESULTS = res
    _unpack_out(y, res.results)
    return y
